# revision 24
# baseline (speedup 1.0000x reference)
"""3-layer GAT on 8 Trainium2 NeuronCores.

Strategy (per sharding hint): destination nodes + incident edges sharded
across 8 cores; weights replicated; per-layer AllGather of node features.

Per layer, per core:
  Phase A (dst side): for each 128-dst chunk, gather the chunk's feature
    rows, transpose on PE, matmul against [W_ad | W_skip] (+bias via ones
    row) -> a_d table (DRAM) + skip rows (SBUF).
  Phase B (edge side): edges sorted by dst, padded per chunk to x128.
    One indirect DMA gathers all source rows of a chunk ([128, T*F]).
    Per 128-edge tile: PE transpose -> xl matmul against
    [W | W@att_src] -> per-edge logits; leaky+exp; one-hot (is_equal of
    iota vs dst_local) aggregation matmul accumulates ex-weighted
    messages + denominators into PSUM; per chunk: normalize, add skip,
    ELU (layer 3: head-mean + log_softmax) -> output rows.

Softmax max-subtraction is dropped: logits here are O(10) so exp() is
exact-safe in fp32, and softmax is shift-invariant so results match the
reference to rounding.
"""
import os
import sys

for _p in ("/opt/trn_rl_repo", "/root/.axon_site/_ro/trn_rl_repo"):
    if os.path.isdir(_p) and _p not in sys.path:
        sys.path.insert(0, _p)

import numpy as np

N0, N1, N2, N3 = 131072, 32768, 8192, 2048
H, C_HID, C_OUT, F_IN = 4, 64, 47, 100
NC_ = 8
P = 128
F32 = None  # set after mybir import


# ---------------------------------------------------------------- host prep

def _build_schedule(src, dst, n_dst, n_src):
    """Per-core edge schedule: edges grouped by 128-dst chunk, then by
    32768-row source bucket (int16 dma_gather range), each bucket run
    padded to x128 slots. Slot k of a chunk = (tile k//128, partition
    k%128). Returns wrapped int16 gather indices + per-chunk-per-bucket
    tile counts (uniform across cores)."""
    nbk = -(-n_src // 32768)
    nd_core = n_dst // NC_
    n_chunks = nd_core // P
    per_core = []
    for c in range(NC_):
        base = c * nd_core
        m = (dst >= base) & (dst < base + nd_core)
        e_src = src[m].astype(np.int64)
        e_dst = (dst[m] - base).astype(np.int64)
        bk = e_src >> 15
        order = np.lexsort((bk, e_dst // P))
        e_src, e_dst, bk = e_src[order], e_dst[order], bk[order]
        counts = np.zeros((n_chunks, nbk), np.int64)
        np.add.at(counts, ((e_dst // P), bk), 1)
        per_core.append((e_src, e_dst, bk, counts))
    counts_all = np.stack([pc[3] for pc in per_core])            # [cores,ch,bk]
    tiles_pcb = -(-counts_all.max(axis=0) // P)                  # [ch, bk]
    # ensure at least one tile per chunk (empty chunks get one bucket-0 tile)
    for i in range(n_chunks):
        if tiles_pcb[i].sum() == 0:
            tiles_pcb[i, 0] = 1
    T_tot = int(tiles_pcb.sum())
    scheds = []
    for c in range(NC_):
        e_src, e_dst, bk, counts = per_core[c]
        idx16 = np.zeros(T_tot * P, dtype=np.int16)
        dloc = np.full(T_tot * P, -1.0, dtype=np.float32)
        adix = np.zeros(T_tot * P, dtype=np.int16)
        s = 0
        epos = 0
        for i in range(n_chunks):
            for b in range(nbk):
                n_e = int(counts[i, b])
                sl = slice(s * P, s * P + n_e)
                idx16[sl] = (e_src[epos:epos + n_e] - (b << 15)).astype(np.int16)
                dloc[sl] = (e_dst[epos:epos + n_e] - i * P).astype(np.float32)
                adix[sl] = e_dst[epos:epos + n_e].astype(np.int16)
                epos += n_e
                s += int(tiles_pcb[i, b])
        assert epos == len(e_src) and s == T_tot
        idxw = np.tile(idx16.reshape(T_tot * 8, 16).T, (8, 1))   # [128, T*8]
        adxw = np.tile(adix.reshape(T_tot * 8, 16).T, (8, 1))
        scheds.append(dict(
            idxw=np.ascontiguousarray(idxw),
            adxw=np.ascontiguousarray(adxw),
            dloc=np.ascontiguousarray(dloc.reshape(T_tot, P).T),
        ))
    return scheds, tiles_pcb.tolist(), n_chunks


def _fold(W, a_s, a_d, b, Ws, bs, Fpad):
    """W_aug [Fpad, cg+4]; W_bigA [Fpad, 4+Cs]; bias row [1, 4+Cs]."""
    h, c = a_s.shape
    F, cg = W.shape
    W_as = np.zeros((F, h), np.float32)
    W_ad = np.zeros((F, h), np.float32)
    for hh in range(h):
        W_as[:, hh] = W[:, hh * c:(hh + 1) * c] @ a_s[hh]
        W_ad[:, hh] = W[:, hh * c:(hh + 1) * c] @ a_d[hh]
    W_aug = np.zeros((Fpad, cg + h), np.float32)
    W_aug[:F, :cg] = W
    W_aug[:F, cg:] = W_as
    Cs = Ws.shape[1]
    W_big = np.zeros((Fpad, h + Cs), np.float32)
    W_big[:F, :h] = W_ad
    W_big[:F, h:] = Ws
    bias = np.zeros((1, h + Cs), np.float32)
    bias[0, h:] = b + bs
    return W_aug, W_big, bias


# ---------------------------------------------------------------- bass build

def _build_nc(cfg):
    from concourse import bass, bacc, mybir, tile
    from concourse.masks import make_identity
    f32 = mybir.dt.float32
    i32 = mybir.dt.int32
    i16 = mybir.dt.int16
    AF = mybir.ActivationFunctionType
    OP = mybir.AluOpType

    nc = bacc.Bacc("TRN2", target_bir_lowering=False, debug=False,
                   num_devices=NC_)

    # ---- I/O declarations
    x_pad = nc.declare_dram_parameter("x_pad", [N0, P], f32, isOutput=False)
    iota_in = nc.declare_dram_parameter("iota_f", [P, P], f32, isOutput=False)
    L = []
    for li, lc in enumerate(cfg["layers"]):
        d = {}
        T_tot, nch = lc["T_tot"], lc["nch"]
        d["idx"] = nc.declare_dram_parameter(f"idx{li}", [P, T_tot * 8], i16, isOutput=False)
        d["dloc"] = nc.declare_dram_parameter(f"dloc{li}", [P, T_tot], f32, isOutput=False)
        d["adix"] = nc.declare_dram_parameter(f"adix{li}", [P, T_tot * 8], i16, isOutput=False)
        d["dstrow"] = nc.declare_dram_parameter(f"dstrow{li}", [P, nch], i32, isOutput=False)
        Fp, Ca, Cpa = lc["Fpad"], lc["Ca"], lc["Cpa"]
        d["waug"] = nc.declare_dram_parameter(f"waug{li}", [Fp, Ca], f32, isOutput=False)
        d["wbig"] = nc.declare_dram_parameter(f"wbig{li}", [Fp, Cpa], f32, isOutput=False)
        d["brow"] = nc.declare_dram_parameter(f"brow{li}", [1, Cpa], f32, isOutput=False)
        L.append(d)
    out_d = nc.declare_dram_parameter("out", [N3 // NC_, C_OUT], f32, isOutput=True)

    with tile.TileContext(nc) as tc:
        with (
            tc.tile_pool(name="const", bufs=1) as constp,
            tc.tile_pool(name="persist", bufs=1) as perp,
            tc.tile_pool(name="g", bufs=2) as gp,
            tc.tile_pool(name="st", bufs=2) as stp,
            tc.tile_pool(name="m", bufs=2) as mp,
            tc.tile_pool(name="gt", bufs=4) as gtp,
            tc.tile_pool(name="small", bufs=2) as smp,
            tc.tile_pool(name="ppa", bufs=2, space="PSUM") as ppa,
            tc.tile_pool(name="ptp", bufs=2, space="PSUM") as ptp,
            tc.tile_pool(name="pxl", bufs=2, space="PSUM") as pxl,
            tc.tile_pool(name="pagg", bufs=2, space="PSUM") as pagg,
            tc.tile_pool(name="dram", bufs=1, space="DRAM") as dramp,
        ):
            ident = constp.tile([P, P], f32, tag="ident")
            make_identity(nc, ident[:])
            iota0 = constp.tile([P, P], f32, tag="iota0")
            nc.sync.dma_start(out=iota0[:], in_=iota_in[:, :])
            iota = constp.tile([P, P], f32, tag="iota")
            nc.vector.tensor_copy(out=iota[:], in_=iota0[:])
            ones = constp.tile([1, P], f32, tag="ones")
            nc.vector.memset(ones[0:1, :], 1.0)

            # persistent per-layer consts
            lt = []
            for li, lc in enumerate(cfg["layers"]):
                Fp, Ca, Cpa, Fk = lc["Fpad"], lc["Ca"], lc["Cpa"], lc["Fk"]
                dd = {}
                dd["waug"] = [constp.tile([P, Ca], f32, tag=f"waug{li}_{k}", name=f"waug{li}_{k}") for k in range(Fk)]
                for k in range(Fk):
                    nc.sync.dma_start(out=dd["waug"][k][:], in_=L[li]["waug"][k * P:(k + 1) * P, :])
                dd["wbig"] = [constp.tile([P, Cpa], f32, tag=f"wbig{li}_{k}", name=f"wbig{li}_{k}") for k in range(Fk)]
                for k in range(Fk):
                    nc.sync.dma_start(out=dd["wbig"][k][:], in_=L[li]["wbig"][k * P:(k + 1) * P, :])
                dd["brow"] = constp.tile([1, Cpa], f32, tag=f"brow{li}", name=f"brow{li}")
                nc.sync.dma_start(out=dd["brow"][0:1, :], in_=L[li]["brow"][0:1, :])
                nch, Cs = lc["nch"], lc["Cs"]
                dd["skip"] = perp.tile([P, nch * Cs], f32, tag=f"skip{li}", name=f"skip{li}")
                dd["adsb"] = perp.tile([P, nch * H], f32, tag=f"adsb{li}", name=f"adsb{li}")
                dd["ad_dram"] = dramp.tile([nch * P, 64], f32, tag=f"ad{li}", name=f"ad{li}")
                lt.append(dd)

            h1s = dramp.tile([N1 // NC_, 256], f32, tag="h1s")
            h1a = dramp.tile([N1, 256], f32, tag="h1a", addr_space="Shared")
            h2s = dramp.tile([N2 // NC_, 256], f32, tag="h2s")
            h2a = dramp.tile([N2, 256], f32, tag="h2a", addr_space="Shared")

            hsrc = [x_pad, h1a, h2a]
            hout = [(h1s, h1a), (h2s, h2a), (None, None)]

            for li, lc in enumerate(cfg["layers"]):
                Fp, Fk, cg, Ca, Cpa = lc["Fpad"], lc["Fk"], lc["cg"], lc["Ca"], lc["Cpa"]
                nch, Cs, tpc, tpcb = lc["nch"], lc["Cs"], lc["tpc"], lc["tpcb"]
                cgh = cg // H
                dd = lt[li]
                src_t = hsrc[li]
                src_ap = src_t[:, :] if li == 0 else src_t[:]

                # ---------------- phase A: dst-side (a_d table + skip rows)
                for i in range(nch):
                    dri = smp.tile([P, 1], i32, tag="dri")
                    nc.sync.dma_start(out=dri[:, 0:1], in_=L[li]["dstrow"][:, i:i + 1])
                    hd0 = gtp.tile([P, Fp], f32, tag="hd0")
                    nc.gpsimd.indirect_dma_start(
                        out=hd0[:, :],
                        out_offset=None,
                        in_=src_ap,
                        in_offset=bass.IndirectOffsetOnAxis(
                            ap=dri[:, 0:1], axis=0),
                    )

                    pa = ppa.tile([P, Cpa], f32, tag="pa", space="PSUM")
                    for k in range(Fk):
                        tp = ptp.tile([P, P], f32, tag="tp", space="PSUM")
                        nc.tensor.transpose(out=tp[:], in_=hd0[:, k * P:(k + 1) * P],
                                            identity=ident[:])
                        ht = gtp.tile([P, P], f32, tag="ht")
                        nc.vector.tensor_copy(out=ht[:], in_=tp[:])
                        nc.tensor.matmul(out=pa[:, :Cpa], lhsT=ht[:],
                                         rhs=dd["wbig"][k][:],
                                         start=(k == 0), stop=False)
                    nc.tensor.matmul(out=pa[:, :Cpa], lhsT=ones[0:1, :],
                                     rhs=dd["brow"][0:1, :], start=False, stop=True)
                    nc.vector.tensor_copy(out=dd["adsb"][:, i * H:(i + 1) * H],
                                          in_=pa[:, 0:H])
                    nc.vector.tensor_copy(out=dd["skip"][:, i * Cs:(i + 1) * Cs],
                                          in_=pa[:, H:H + Cs])
                # a_d table to DRAM: [nd, 4] <- [P, nch, 4]
                ad_view = dd["ad_dram"][:].rearrange("(c p) f -> p c f", p=P)[:, :, 0:H]
                nc.sync.dma_start(out=ad_view,
                                  in_=dd["adsb"][:].rearrange("p (c f) -> p c f", f=H))

                # ---------------- phase B: edges
                T_MAX = max(tpc)
                t0 = 0
                for i in range(nch):
                    T = tpc[i]
                    ixw = smp.tile([P, T_MAX * 8], i16, tag="ixw")
                    nc.sync.dma_start(out=ixw[:, :T * 8],
                                      in_=L[li]["idx"][:, 8 * t0:8 * (t0 + T)])
                    axw = smp.tile([P, T_MAX * 8], i16, tag="axw")
                    nc.sync.dma_start(out=axw[:, :T * 8],
                                      in_=L[li]["adix"][:, 8 * t0:8 * (t0 + T)])
                    g0 = gp.tile([P, T_MAX * Fp], f32, tag="g0")
                    off = 0
                    n_rows = [N0, N1, N2][li]
                    for b in range(len(tpcb[i])):
                        Tb = tpcb[i][b]
                        if Tb == 0:
                            continue
                        lo = b * 32768
                        hi = min(lo + 32768, n_rows)
                        src_sl = (src_t[lo:hi, :] if li == 0
                                  else src_t[:][lo:hi, :])
                        for s0 in range(0, Tb, 4):
                            sn = min(4, Tb - s0)
                            o2 = off + s0
                            nc.gpsimd.dma_gather(
                                out_ap=g0[:, o2 * Fp:(o2 + sn) * Fp].rearrange(
                                    "p (j r) -> p j r", r=Fp),
                                in_ap=src_sl,
                                idxs_ap=ixw[:, 8 * o2:8 * (o2 + sn)],
                                num_idxs=sn * P, num_idxs_reg=sn * P,
                                elem_size=Fp, single_packet=False)
                        off += Tb
                    adg0 = smp.tile([P, T_MAX * 64], f32, tag="adg0")
                    for s0 in range(0, T, 8):
                        sn = min(8, T - s0)
                        nc.gpsimd.dma_gather(
                            out_ap=adg0[:, s0 * 64:(s0 + sn) * 64].rearrange(
                                "p (j r) -> p j r", r=64),
                            in_ap=dd["ad_dram"][:],
                            idxs_ap=axw[:, 8 * s0:8 * (s0 + sn)],
                            num_idxs=sn * P, num_idxs_reg=sn * P,
                            elem_size=64, single_packet=False)
                    dl = smp.tile([P, T_MAX], f32, tag="dl")
                    nc.sync.dma_start(out=dl[:, :T], in_=L[li]["dloc"][:, t0:t0 + T])
                    dl2 = smp.tile([P, T_MAX], f32, tag="dl2")
                    nc.vector.tensor_copy(out=dl2[:, :T], in_=dl[:, :T])
                    # one-hot S_T [e, d] for all tiles of the chunk
                    st = stp.tile([P, T_MAX * P], f32, tag="st")
                    nc.vector.tensor_tensor(
                        out=st[:, :T * P].rearrange("p (t d) -> p t d", d=P),
                        in0=iota[:].rearrange("p (o d) -> p o d", o=1).to_broadcast([P, T, P]),
                        in1=dl2[:, :T].rearrange("p (t o) -> p t o", o=1).to_broadcast([P, T, P]),
                        op=OP.is_equal,
                    )
                    m_all = mp.tile([P, T_MAX * Ca], f32, tag="m")
                    nc.vector.memset(m_all[:, 0:1], 0.0)
                    tb = smp.tile([P, T_MAX * H], f32, tag="tb")
                    lk = smp.tile([P, T_MAX * H], f32, tag="lk")
                    agg = pagg.tile([P, Ca], f32, tag="agg", space="PSUM")
                    for t in range(T):
                        xl = pxl.tile([P, Ca], f32, tag="xl", space="PSUM")
                        for k in range(Fk):
                            tp = ptp.tile([P, P], f32, tag="tp", space="PSUM")
                            nc.tensor.transpose(
                                out=tp[:], in_=g0[:, t * Fp + k * P: t * Fp + (k + 1) * P],
                                identity=ident[:])
                            gt = gtp.tile([P, P], f32, tag="gt")
                            nc.vector.tensor_copy(out=gt[:], in_=tp[:])
                            nc.tensor.matmul(out=xl[:, :Ca], lhsT=gt[:],
                                             rhs=dd["waug"][k][:],
                                             start=(k == 0), stop=(k == Fk - 1))
                        # logits = leaky(a_s(src) + a_d(dst)); ex into M cols cg:cg+4
                        ts_ = slice(t * H, (t + 1) * H)
                        nc.vector.tensor_tensor(
                            out=tb[:, ts_], in0=xl[:, cg:cg + H],
                            in1=adg0[:, t * 64:t * 64 + H], op=OP.add)
                        nc.vector.tensor_scalar(out=lk[:, ts_], in0=tb[:, ts_],
                                                scalar1=0.2, scalar2=None, op0=OP.mult)
                        nc.vector.tensor_tensor(out=lk[:, ts_], in0=lk[:, ts_],
                                                in1=tb[:, ts_], op=OP.max)
                        exb = smp.tile([P, H], f32, tag="exb")
                        nc.scalar.activation(out=exb[:, :], in_=lk[:, ts_],
                                             func=AF.Exp)
                        nc.vector.tensor_copy(
                            out=m_all[:, t * Ca + cg:t * Ca + cg + H],
                            in_=exb[:, :])
                        # M[:, :cg] = xl * ex (per-head broadcast)
                        nc.vector.tensor_tensor(
                            out=m_all[:, t * Ca:t * Ca + cg].rearrange(
                                "p (h c) -> p h c", c=cgh),
                            in0=xl[:, 0:cg].rearrange("p (h c) -> p h c", c=cgh),
                            in1=m_all[:, t * Ca + cg:t * Ca + cg + H].rearrange(
                                "p (h o) -> p h o", o=1).to_broadcast([P, H, cgh]),
                            op=OP.mult)
                        nc.tensor.matmul(out=agg[:, :Ca],
                                         lhsT=st[:, t * P:(t + 1) * P],
                                         rhs=m_all[:, t * Ca:(t + 1) * Ca],
                                         start=(t == 0), stop=(t == T - 1))
                    # ---- finalize chunk
                    rc = smp.tile([P, H], f32, tag="rc")
                    nc.vector.reciprocal(out=rc[:, :], in_=agg[:, cg:cg + H])
                    if li < 2:
                        ho = gp.tile([P, cg], f32, tag="ho")
                        nc.vector.memset(ho[:, 0:1], 0.0)
                        for hh in range(H):
                            nc.vector.tensor_scalar(
                                out=ho[:, hh * cgh:(hh + 1) * cgh],
                                in0=agg[:, hh * cgh:(hh + 1) * cgh],
                                scalar1=rc[:, hh:hh + 1], scalar2=None, op0=OP.mult)
                        nc.vector.tensor_tensor(out=ho[:], in0=ho[:],
                                                in1=dd["skip"][:, i * Cs:(i + 1) * Cs],
                                                op=OP.add)
                        # ELU = relu(x) + exp(min(x,0)) - 1
                        mn = gp.tile([P, cg], f32, tag="mn")
                        nc.vector.tensor_scalar(out=mn[:], in0=ho[:], scalar1=0.0,
                                                scalar2=None, op0=OP.min)
                        nc.scalar.activation(out=mn[:], in_=mn[:], func=AF.Exp)
                        nc.scalar.activation(out=ho[:], in_=ho[:], func=AF.Relu)
                        nc.vector.tensor_tensor(out=ho[:], in0=ho[:], in1=mn[:], op=OP.add)
                        nc.vector.tensor_scalar(out=ho[:], in0=ho[:], scalar1=-1.0,
                                                scalar2=None, op0=OP.add)
                        hs = hout[li][0]
                        nc.sync.dma_start(out=hs[i * P:(i + 1) * P, :], in_=ho[:])
                    else:
                        # head mean + skip + log_softmax
                        hm = smp.tile([P, 4 * C_OUT], f32, tag="hm")
                        nc.vector.memset(hm[:, 0:1], 0.0)
                        for hh in range(H):
                            nc.vector.tensor_scalar(
                                out=hm[:, hh * C_OUT:(hh + 1) * C_OUT],
                                in0=agg[:, hh * cgh:(hh + 1) * cgh],
                                scalar1=rc[:, hh:hh + 1], scalar2=0.25,
                                op0=OP.mult, op1=OP.mult)
                        ho = smp.tile([P, C_OUT], f32, tag="ho3")
                        nc.vector.tensor_tensor(out=ho[:], in0=hm[:, 0:C_OUT],
                                                in1=hm[:, C_OUT:2 * C_OUT], op=OP.add)
                        nc.vector.tensor_tensor(out=ho[:], in0=ho[:],
                                                in1=hm[:, 2 * C_OUT:3 * C_OUT], op=OP.add)
                        nc.vector.tensor_tensor(out=ho[:], in0=ho[:],
                                                in1=hm[:, 3 * C_OUT:4 * C_OUT], op=OP.add)
                        nc.vector.tensor_tensor(out=ho[:], in0=ho[:],
                                                in1=dd["skip"][:, i * Cs:(i + 1) * Cs],
                                                op=OP.add)
                        mx = smp.tile([P, 1], f32, tag="mx")
                        nc.vector.tensor_reduce(out=mx[:, 0:1], in_=ho[:],
                                                axis=mybir.AxisListType.X, op=OP.max)
                        z = smp.tile([P, C_OUT], f32, tag="z")
                        nc.vector.tensor_scalar(out=z[:], in0=ho[:],
                                                scalar1=mx[:, 0:1], scalar2=None,
                                                op0=OP.subtract)
                        ez = smp.tile([P, C_OUT], f32, tag="ez")
                        nc.scalar.activation(out=ez[:], in_=z[:], func=AF.Exp)
                        sm = smp.tile([P, 1], f32, tag="sm")
                        nc.vector.tensor_reduce(out=sm[:, 0:1], in_=ez[:],
                                                axis=mybir.AxisListType.X, op=OP.add)
                        ln = smp.tile([P, 1], f32, tag="ln")
                        nc.scalar.activation(out=ln[:, 0:1], in_=sm[:, 0:1], func=AF.Ln)
                        ln2 = smp.tile([P, 1], f32, tag="ln2")
                        nc.vector.tensor_copy(out=ln2[:, 0:1], in_=ln[:, 0:1])
                        zo = smp.tile([P, C_OUT], f32, tag="zo")
                        nc.vector.memset(zo[:, 0:1], 0.0)
                        nc.vector.tensor_scalar(out=zo[:], in0=z[:],
                                                scalar1=ln2[:, 0:1], scalar2=None,
                                                op0=OP.subtract)
                        nc.sync.dma_start(out=out_d[i * P:(i + 1) * P, :], in_=zo[:])
                    t0 += T

                # ---------------- all-gather H for next layer
                if li < 2:
                    hs, ha = hout[li]
                    nc.gpsimd.collective_compute(
                        "AllGather",
                        mybir.AluOpType.bypass,
                        replica_groups=[list(range(NC_))],
                        ins=[hs[:].opt()],
                        outs=[ha[:].opt()],
                    )
    nc.compile()
    return nc


# ---------------------------------------------------------------- entry

def kernel(**inputs):
    out, _ = run(inputs, trace=False)
    return out


def run(inputs, trace=False):
    from concourse import bass_utils

    x = inputs["x"].astype(np.float32)
    x_pad = np.zeros((N0, P), np.float32)
    x_pad[:, :F_IN] = x

    sch1, tpcb1, nch1 = _build_schedule(inputs["src1"], inputs["dst1"], N1, N0)
    sch2, tpcb2, nch2 = _build_schedule(inputs["src2"], inputs["dst2"], N2, N1)
    sch3, tpcb3, nch3 = _build_schedule(inputs["src3"], inputs["dst3"], N3, N2)
    tpc1 = [sum(r) for r in tpcb1]
    tpc2 = [sum(r) for r in tpcb2]
    tpc3 = [sum(r) for r in tpcb3]

    Waug1, Wbig1, brow1 = _fold(inputs["W1"], inputs["as1"], inputs["ad1"],
                                inputs["b1"], inputs["Ws1"], inputs["bs1"], 128)
    Waug2, Wbig2, brow2 = _fold(inputs["W2"], inputs["as2"], inputs["ad2"],
                                inputs["b2"], inputs["Ws2"], inputs["bs2"], 256)
    Waug3, Wbig3, brow3 = _fold(inputs["W3"], inputs["as3"], inputs["ad3"],
                                inputs["b3"], inputs["Ws3"], inputs["bs3"], 256)

    cfg = {"layers": [
        dict(T_tot=sum(tpc1), nch=nch1, tpc=tpc1, tpcb=tpcb1, Fpad=128, Fk=1,
             cg=256, Ca=260, Cpa=260, Cs=256),
        dict(T_tot=sum(tpc2), nch=nch2, tpc=tpc2, tpcb=tpcb2, Fpad=256, Fk=2,
             cg=256, Ca=260, Cpa=260, Cs=256),
        dict(T_tot=sum(tpc3), nch=nch3, tpc=tpc3, tpcb=tpcb3, Fpad=256, Fk=2,
             cg=188, Ca=192, Cpa=51, Cs=47),
    ]}

    nc = _build_nc(cfg)

    iota_f = np.tile(np.arange(P, dtype=np.float32)[None, :], (P, 1))
    in_maps = []
    for c in range(NC_):
        m = {
            "x_pad": x_pad, "iota_f": iota_f,
            "waug0": Waug1, "wbig0": Wbig1, "brow0": brow1,
            "waug1": Waug2, "wbig1": Wbig2, "brow1": brow2,
            "waug2": Waug3, "wbig2": Wbig3, "brow2": brow3,
        }
        for li, (sch, nch, ndst) in enumerate(
                [(sch1, nch1, N1), (sch2, nch2, N2), (sch3, nch3, N3)]):
            s = sch[c]
            m[f"idx{li}"] = s["idxw"]
            m[f"dloc{li}"] = s["dloc"]
            m[f"adix{li}"] = s["adxw"]
            base = c * (ndst // NC_)
            m[f"dstrow{li}"] = np.ascontiguousarray(
                (base + np.arange(nch)[None, :] * P
                 + np.arange(P)[:, None]).astype(np.int32))
        in_maps.append(m)

    if trace:
        out, times = _bench_pjrt(nc, in_maps, iters=4)
        return out, times
    res = bass_utils.run_bass_kernel_spmd(nc, in_maps, list(range(NC_)),
                                          trace=False)
    out = np.concatenate([res.results[c]["out"] for c in range(NC_)], axis=0)
    return out.astype(np.float32), res


def _bench_pjrt(nc, in_maps, iters=4):
    """Mirror bass2jax.run_bass_via_pjrt multi-core path, but keep inputs
    device-resident and time repeated executions (min wall over iters)."""
    import time
    import jax
    from jax.sharding import Mesh, PartitionSpec, NamedSharding
    from jax.experimental.shard_map import shard_map
    from concourse import bass2jax, mybir

    bass2jax.install_neuronx_cc_hook()
    pid_name = nc.partition_id_tensor.name if nc.partition_id_tensor else None
    in_names, out_names, out_avals, zero_outs = [], [], [], []
    for alloc in nc.m.functions[0].allocations:
        if not isinstance(alloc, mybir.MemoryLocationSet):
            continue
        name = alloc.memorylocations[0].name
        if alloc.kind == "ExternalInput":
            if name != pid_name:
                in_names.append(name)
        elif alloc.kind == "ExternalOutput":
            out_names.append(name)
            shape = tuple(alloc.tensor_shape)
            dtype = mybir.dt.np(alloc.dtype)
            out_avals.append(jax.core.ShapedArray(shape, dtype))
            zero_outs.append(np.zeros(shape, dtype))
    n_params = len(in_names)
    all_names = in_names + out_names
    if pid_name is not None:
        all_names = all_names + [pid_name]

    def _body(*args):
        operands = list(args)
        if pid_name is not None:
            operands.append(bass2jax.partition_id_tensor())
        outs = bass2jax._bass_exec_p.bind(
            *operands, out_avals=tuple(out_avals), in_names=tuple(all_names),
            out_names=tuple(out_names), lowering_input_output_aliases=(),
            sim_require_finite=True, sim_require_nnan=True, nc=nc)
        return tuple(outs)

    devices = jax.devices()[:NC_]
    mesh = Mesh(np.asarray(devices), ("core",))
    in_specs = (PartitionSpec("core"),) * (n_params + len(out_names))
    out_specs = (PartitionSpec("core"),) * len(out_names)
    donate = tuple(range(n_params, n_params + len(out_names)))
    sharded = jax.jit(
        shard_map(_body, mesh=mesh, in_specs=in_specs, out_specs=out_specs,
                  check_rep=False),
        donate_argnums=donate, keep_unused=True)
    sh = NamedSharding(mesh, PartitionSpec("core"))
    concat_in = [
        jax.device_put(
            np.concatenate([np.asarray(in_maps[c][n]) for c in range(NC_)],
                           axis=0), sh)
        for n in in_names]
    times = []
    out_arrs = None
    for _ in range(iters):
        concat_zeros = [
            np.zeros((NC_ * z.shape[0], *z.shape[1:]), z.dtype)
            for z in zero_outs]
        t0 = time.time()
        out_arrs = sharded(*concat_in, *concat_zeros)
        jax.block_until_ready(out_arrs)
        times.append(time.time() - t0)
    i = out_names.index("out")
    full = np.asarray(out_arrs[i])
    out = full.reshape(NC_, -1, full.shape[-1]).reshape(-1, full.shape[-1])
    return out.astype(np.float32), times
